# revision 34
# baseline (speedup 1.0000x reference)
"""Embedding lookup (nn_LookupNetwork) on 8 Trainium2 NeuronCores.

Strategy: data-parallel over the batch; each core handles 512 of the 4096
batch rows (102,400 lookups) with a replicated table in local HBM.

The gather uses the Q7 `dma_gather` SWDGE instruction (one descriptor per
looked-up row, streamed through all 16 SDMA engines) instead of per-column
indirect DMAs (~70x faster here). dma_gather indices are int16, so the
100k-row table is re-laid-out on the host into 4 chunks of 32256 rows +
512 zero rows each (device table [4*32768, 128] bf16); every position is
gathered once per chunk, with out-of-chunk positions (and the -1 sentinel)
pointed at a zero row CHOSEN BY SLOT ID - funneling them all to a single
zero row is a one-address HBM hotspot that collapses gather bandwidth ~10x
across the 8 cores. The 4 chunk gathers go out on the 4 SWDGE queues (one
queue is Q7-descgen-bound). The 4 gathered tiles are summed on the DVE
(tensor_tensor only - copy/cast-class DVE ops enter 2-port SBUF mode and
stall SWDGE descgen); at most one term per position is nonzero, so bf16
sums are exact and the last add emits the f32 output tile. Only the table
quantization to bf16 loses bits (rel err ~3e-3, gate is 2e-2), and it
halves both gather traffic and DVE time.

dma_gather writes element i to partition i%128, block i//128; the host
pre-permutes the index slots so that the summed SBUF tile is already in
natural row order per partition, making the output store a contiguous
128 x 16KB HWDGE DMA and the host-side unshard a pure reshape.
"""

import sys

sys.path.insert(0, "/opt/trn_rl_repo")

from contextlib import ExitStack

import numpy as np

import concourse.bacc as bacc
import concourse.bass as bass
import concourse.mybir as mybir
import concourse.tile as tile
from concourse.bass_utils import run_bass_kernel_spmd

VOCAB, D = 100000, 128
BATCH, HIST = 4096, 200
NCORES = 8
P = 128

V_CH = 32256            # data rows per chunk
ZR = 512                # zero rows per chunk; out-of-chunk positions are
                        # spread across them (a single shared zero row is an
                        # HBM hotspot that collapses gather bandwidth ~10x)
NCH = 4                 # chunks needed to cover VOCAB
TBL_CH = V_CH + ZR      # 32768 device-table rows per chunk
S = 4096                # positions gathered per tile
BLK = S // P            # output rows per partition per tile
NPOS = BATCH * HIST // NCORES   # 102400 positions per core
NT = NPOS // S          # tiles per core

_nc_cache = {}


def build_nc(nt=NT, bufs=2, reps=1, mode="full", bf16=False):
    """reps>1 repeats the whole workload in-program (for timing: the
    per-exec HW time is (t(reps=R) - t(reps=1)) / (R - 1), which cancels
    the host/axon dispatch overhead). mode: full | gathers | g+adds |
    g+store (ablations for perf attribution). bf16: gather/combine in
    bf16 (each position has at most one nonzero term across chunks, so
    the bf16 adds are exact; only the table quantization loses bits)."""
    dt = mybir.dt.bfloat16 if bf16 else mybir.dt.float32
    nc = bacc.Bacc(
        "TRN2", target_bir_lowering=False, debug=False, enable_asserts=False,
        num_swdge_queues=4,
    )
    idx_d = nc.dram_tensor(
        "idx", [P, nt * NCH * (S // 16)], mybir.dt.int16, kind="ExternalInput"
    ).ap()
    tab_d = nc.dram_tensor(
        "table", [NCH * TBL_CH, D], dt, kind="ExternalInput"
    ).ap()
    out_d = nc.dram_tensor(
        "out", [nt, P, S], mybir.dt.float32, kind="ExternalOutput"
    ).ap()

    cols_t = NCH * (S // 16)        # idx columns per tile (all 4 chunks)
    with tile.TileContext(nc) as tc:
        with ExitStack() as ctx:
            ipool = ctx.enter_context(tc.tile_pool(name="ipool", bufs=3))
            gpools = [
                ctx.enter_context(tc.tile_pool(name=f"g{c}", bufs=bufs))
                for c in range(NCH)
            ]
            opool = (
                ctx.enter_context(tc.tile_pool(name="opool", bufs=bufs))
                if bf16 else None
            )
            for t in [t for _ in range(reps) for t in range(nt)]:
                idx_t = ipool.tile([P, cols_t], mybir.dt.int16)
                nc.scalar.dma_start(
                    idx_t[:], idx_d[:, t * cols_t : (t + 1) * cols_t]
                )
                gs = []
                for c in range(NCH):
                    g = gpools[c].tile([P, S], dt)
                    g3 = g[:].rearrange("p (b d) -> p b d", d=D)
                    nc.gpsimd.dma_gather(
                        g3,
                        tab_d[c * TBL_CH : (c + 1) * TBL_CH, :],
                        idx_t[:, c * (S // 16) : (c + 1) * (S // 16)],
                        S,
                        S,
                        D,
                        # >64 descriptors per lane per instruction cannot be
                        # coalesced into one SDMA packet
                        single_packet=False,
                        # one SWDGE queue is Q7-descgen-bound (~27us per
                        # 4096-idx gather); spread chunks over all 4 queues
                        queue_num=c,
                    )
                    gs.append(g)
                if mode in ("full", "g+adds"):
                    nc.vector.tensor_tensor(
                        out=gs[0][:], in0=gs[0][:], in1=gs[1][:],
                        op=mybir.AluOpType.add,
                    )
                    nc.vector.tensor_tensor(
                        out=gs[2][:], in0=gs[2][:], in1=gs[3][:],
                        op=mybir.AluOpType.add,
                    )
                    if bf16:
                        final = opool.tile([P, S], mybir.dt.float32)
                        nc.vector.tensor_tensor(
                            out=final[:], in0=gs[0][:], in1=gs[2][:],
                            op=mybir.AluOpType.add,
                        )
                    else:
                        final = gs[0]
                        nc.vector.tensor_tensor(
                            out=final[:], in0=gs[0][:], in1=gs[2][:],
                            op=mybir.AluOpType.add,
                        )
                if mode in ("full", "g+store"):
                    nc.sync.dma_start(out_d[t], final[:])
    nc.compile()
    return nc


V4_ZR = 128             # spread sentinel zero-elements (hotspot avoidance)
V4_Q = 25000 + V4_ZR    # gather elements of 4 rows each
V4_ROWS = 4 * V4_Q


def build_nc_mod4(nt=NT, bufs=2, reps=1, mode="full", select="mult"):
    """mod-4 variant: ONE dma_gather per tile in bf16 at elem=1024B
    (4 consecutive table rows per descriptor, idx16 = row>>2 < 25001),
    then a 4-way quarter-select on the DVE with host-precomputed one-hot
    masks (the -1 sentinel points at the appended zero rows). 105 MB
    gathered per core instead of 210 MB, and 4x fewer Q7-generated
    descriptors. select: "cp" = copy + 3 copy_predicated (4 DVE passes);
    "mult" = mask-multiply-accumulate tree (7 passes, CoreSim-friendly)."""
    nc = bacc.Bacc(
        "TRN2", target_bir_lowering=False, debug=False, enable_asserts=False,
        num_swdge_queues=4,
    )
    cols_i = S // 16                # 256 idx cols
    cols_m = 4 * BLK                # masks m3|m2|m1|m0 as bf16 bit patterns
    cols_t = cols_i + cols_m
    idx_d = nc.dram_tensor(
        "idx", [P, nt * cols_t], mybir.dt.int16, kind="ExternalInput"
    ).ap()
    tab_d = nc.dram_tensor(
        "table", [V4_Q, 512], mybir.dt.bfloat16, kind="ExternalInput"
    ).ap()
    out_d = nc.dram_tensor(
        "out", [nt, P, S], mybir.dt.float32, kind="ExternalOutput"
    ).ap()

    with tile.TileContext(nc) as tc:
        with ExitStack() as ctx:
            ipool = ctx.enter_context(tc.tile_pool(name="ipool", bufs=3))
            gpool = ctx.enter_context(tc.tile_pool(name="gpool", bufs=bufs))
            opool = ctx.enter_context(tc.tile_pool(name="opool", bufs=bufs))
            tpool = ctx.enter_context(tc.tile_pool(name="tpool", bufs=bufs))
            for i, t in enumerate([t for _ in range(reps) for t in range(nt)]):
                im_t = ipool.tile([P, cols_t], mybir.dt.int16)
                nc.scalar.dma_start(
                    im_t[:], idx_d[:, t * cols_t : (t + 1) * cols_t]
                )
                g = gpool.tile([P, BLK * 512], mybir.dt.bfloat16)
                g3 = g[:].rearrange("p (b e) -> p b e", e=512)
                nc.gpsimd.dma_gather(
                    g3, tab_d, im_t[:, :cols_i], S, S, 512,
                    single_packet=False, queue_num=i % 4,
                )
                if mode == "gathers":
                    continue

                def mk(r):
                    # mask r as bf16 broadcast over the embedding dim
                    k = 3 - r
                    return (
                        im_t[:, cols_i + k * BLK : cols_i + (k + 1) * BLK]
                        .bitcast(mybir.dt.bfloat16)
                        .unsqueeze(2)
                        .to_broadcast([P, BLK, D])
                    )

                def gq(r):
                    return g3[:, :, r * D : (r + 1) * D]

                o = opool.tile([P, S], mybir.dt.float32)
                o3 = o[:].rearrange("p (b d) -> p b d", d=D)
                if select == "cp":
                    nc.vector.tensor_copy(out=o3, in_=gq(3))
                    for r in (2, 1, 0):
                        nc.vector.copy_predicated(o3, mk(r), gq(r))
                else:
                    tmp = tpool.tile([P, S], mybir.dt.float32)
                    t3 = tmp[:].rearrange("p (b d) -> p b d", d=D)
                    nc.vector.tensor_tensor(
                        out=o3, in0=gq(3), in1=mk(3), op=mybir.AluOpType.mult
                    )
                    for r in (2, 1, 0):
                        nc.vector.tensor_tensor(
                            out=t3, in0=gq(r), in1=mk(r),
                            op=mybir.AluOpType.mult,
                        )
                        nc.vector.tensor_tensor(
                            out=o3, in0=o3, in1=t3, op=mybir.AluOpType.add
                        )
                if mode in ("full", "g+store"):
                    nc.sync.dma_start(out_d[t], o[:])
    nc.compile()
    return nc


def _in_maps_mod4(input_batch, table, nt=NT):
    import ml_dtypes

    v = np.asarray(input_batch).astype(np.int64).reshape(NCORES, NT, P, BLK)
    v = v[:, :nt]
    posid = (np.arange(P)[:, None] * BLK + np.arange(BLK)[None, :]) & (V4_ZR - 1)
    q = np.where(v >= 0, v >> 2, 25000 + posid[None, None]).astype(np.int16)
    r = np.where(v >= 0, v & 3, 3)
    # idx slots: tile-transposed + wrap-16 (same as the chunked variant)
    slots = q.transpose(0, 1, 3, 2).reshape(NCORES, nt, S)
    w = slots.reshape(NCORES, nt, S // 16, 16).transpose(0, 1, 3, 2)  # [c,nt,16,256]
    idx16 = np.tile(w.reshape(NCORES, nt, 16, S // 16), (1, 1, 8, 1))  # [c,nt,128,256]
    # masks in natural [p, b] layout, one-hot bf16 bit patterns as int16
    masks = [
        (r == rr).astype(ml_dtypes.bfloat16).view(np.int16) for rr in (3, 2, 1, 0)
    ]  # each [c, nt, 128, BLK]
    im = np.concatenate([idx16] + masks, axis=3)  # [c, nt, 128, 256+128]
    im = im.transpose(0, 2, 1, 3).reshape(NCORES, P, nt * (S // 16 + 4 * BLK))
    im = np.ascontiguousarray(im)

    tab = np.asarray(table, dtype=np.float32)
    t4 = np.zeros((V4_ROWS, D), ml_dtypes.bfloat16)
    t4[:VOCAB] = tab.astype(ml_dtypes.bfloat16)
    t4 = np.ascontiguousarray(t4.reshape(V4_Q, 512))
    return [{"idx": im[c], "table": t4} for c in range(NCORES)]


def build_raw(nt=NT, reps=1):
    """Raw Block-mode version of the chunked kernel (no TileContext): the
    tile-scheduled build ran gathers ~30x slower than the identical raw
    instruction stream, so the pipeline semaphores are written by hand.

    Pipeline (per global iteration i, tile t = i % nt):
      ACT   : idx load t (HWDGE, 3 bufs)       -> s_idx += 16
      GPSIMD: 4x dma_gather (queues 0-3, 2 bufs/chunk) -> s_g[c] += 16
      DVE   : g0+=g1, g2+=g3, g0+=g2 (tensor_tensor only; copy/cast DVE
              ops would 2-port-block SWDGE)    -> s_dve += 1 each
      SP    : store g0 -> out[t] (HWDGE)       -> s_st += 16
    """
    from concourse.library_config import mlp

    nc = bacc.Bacc(
        "TRN2", target_bir_lowering=False, debug=False, enable_asserts=False,
        num_swdge_queues=4,
    )
    cols_t = NCH * (S // 16)
    idx_d = nc.dram_tensor(
        "idx", [P, nt * cols_t], mybir.dt.int16, kind="ExternalInput"
    ).ap()
    tab_d = nc.dram_tensor(
        "table", [NCH * TBL_CH, D], mybir.dt.float32, kind="ExternalInput"
    ).ap()
    out_d = nc.dram_tensor(
        "out", [nt, P, S], mybir.dt.float32, kind="ExternalOutput"
    ).ap()

    IB = 3          # idx buffers
    GB = 2          # gather buffers per chunk
    n = reps * nt

    with nc.Block() as block, ExitStack() as st:
        ibufs = [
            st.enter_context(nc.sbuf_tensor(f"ib{k}", [P, cols_t], mybir.dt.int16))
            for k in range(IB)
        ]
        gbufs = [
            [
                st.enter_context(
                    nc.sbuf_tensor(f"g{c}_{b}", [P, S], mybir.dt.float32)
                )
                for b in range(GB)
            ]
            for c in range(NCH)
        ]
        # per-buffer sems: parallel DMA completions are not ordered across
        # buffers, so each (chunk, buf) / idx-buf / store-buf gets its own
        s_ib = [st.enter_context(nc.semaphore(f"s_ib{k}")) for k in range(IB)]  # noqa: ANT232
        s_g = [
            [st.enter_context(nc.semaphore(f"s_g{c}_{b}")) for b in range(GB)]  # noqa: ANT232
            for c in range(NCH)
        ]
        s_dve = st.enter_context(nc.semaphore("s_dve"))
        s_st = [st.enter_context(nc.semaphore(f"s_st{b}")) for b in range(GB)]  # noqa: ANT232

        @block.scalar
        def _(act):
            for i in range(n):
                t = i % nt
                if i >= IB:
                    j = i - IB
                    for c in range(NCH):
                        act.wait_ge(s_g[c][j % GB], 16 * (j // GB + 1))
                act.dma_start(
                    ibufs[i % IB][:, :], idx_d[:, t * cols_t : (t + 1) * cols_t]
                ).then_inc(s_ib[i % IB], 16)

        @block.gpsimd
        def _(gpsimd):
            gpsimd.load_library(mlp)
            for i in range(n):
                gpsimd.wait_ge(s_ib[i % IB], 16 * (i // IB + 1))
                if i >= GB:
                    j = i - GB
                    gpsimd.wait_ge(s_st[j % GB], 16 * (j // GB + 1))
                    gpsimd.wait_ge(s_dve, 3 * (j + 1))
                for c in range(NCH):
                    g3 = gbufs[c][i % GB][:].rearrange("p (b d) -> p b d", d=D)
                    gpsimd.dma_gather(
                        g3,
                        tab_d[c * TBL_CH : (c + 1) * TBL_CH, :],
                        ibufs[i % IB][:, c * (S // 16) : (c + 1) * (S // 16)],
                        S,
                        S,
                        D,
                        single_packet=False,
                        queue_num=c,
                    ).then_inc(s_g[c][i % GB], 16)

        @block.vector
        def _(vector):
            for i in range(n):
                b = i % GB
                r = 16 * (i // GB + 1)
                vector.wait_ge(s_g[0][b], r)
                vector.wait_ge(s_g[1][b], r)
                vector.tensor_tensor(
                    out=gbufs[0][b][:], in0=gbufs[0][b][:], in1=gbufs[1][b][:],
                    op=mybir.AluOpType.add,
                ).then_inc(s_dve, 1)
                vector.wait_ge(s_g[2][b], r)
                vector.wait_ge(s_g[3][b], r)
                vector.tensor_tensor(
                    out=gbufs[2][b][:], in0=gbufs[2][b][:], in1=gbufs[3][b][:],
                    op=mybir.AluOpType.add,
                ).then_inc(s_dve, 1)
                # same-engine order already guarantees this; explicit wait
                # keeps the CoreSim race detector happy
                vector.wait_ge(s_dve, 3 * i + 2)
                vector.tensor_tensor(
                    out=gbufs[0][b][:], in0=gbufs[0][b][:], in1=gbufs[2][b][:],
                    op=mybir.AluOpType.add,
                ).then_inc(s_dve, 1)

        @block.sync
        def _(sync):
            for i in range(n):
                t = i % nt
                sync.wait_ge(s_dve, 3 * (i + 1))
                sync.dma_start(out_d[t], gbufs[0][i % GB][:]).then_inc(
                    s_st[i % GB], 16
                )
            for b in range(GB):
                cnt = len([i for i in range(n) if i % GB == b])
                sync.wait_ge(s_st[b], 16 * cnt)

    nc.compile()
    return nc


def build_raw_mod4(nt=NT, reps=1):
    """Raw Block-mode mod-4 kernel: one bf16 dma_gather per tile
    (elem=1024B = 4 rows, idx16 = row>>2), quarter-select as a
    mask-multiply-accumulate tree of tensor_tensor ops (tensor_tensor
    never enters DVE 2-port mode, so it cannot stall SWDGE descgen)."""
    from concourse.library_config import mlp

    nc = bacc.Bacc(
        "TRN2", target_bir_lowering=False, debug=False, enable_asserts=False,
        num_swdge_queues=4,
    )
    cols_i = S // 16
    cols_m = 4 * BLK
    cols_t = cols_i + cols_m
    idx_d = nc.dram_tensor(
        "idx", [P, nt * cols_t], mybir.dt.int16, kind="ExternalInput"
    ).ap()
    tab_d = nc.dram_tensor(
        "table", [V4_Q, 512], mybir.dt.bfloat16, kind="ExternalInput"
    ).ap()
    out_d = nc.dram_tensor(
        "out", [nt, P, S], mybir.dt.float32, kind="ExternalOutput"
    ).ap()

    IB, GB, OB = 3, 3, 2
    n = reps * nt

    with nc.Block() as block, ExitStack() as st:
        ibufs = [
            st.enter_context(nc.sbuf_tensor(f"ib{k}", [P, cols_t], mybir.dt.int16))
            for k in range(IB)
        ]
        gbufs = [
            st.enter_context(
                nc.sbuf_tensor(f"g{b}", [P, BLK * 512], mybir.dt.bfloat16)
            )
            for b in range(GB)
        ]
        obufs = [
            st.enter_context(nc.sbuf_tensor(f"o{b}", [P, S], mybir.dt.float32))
            for b in range(OB)
        ]
        tbuf = st.enter_context(nc.sbuf_tensor("tmp", [P, S], mybir.dt.float32))
        s_ib = [st.enter_context(nc.semaphore(f"s_ib{k}")) for k in range(IB)]  # noqa: ANT232
        s_g = [st.enter_context(nc.semaphore(f"s_g{b}")) for b in range(GB)]  # noqa: ANT232
        s_dve = st.enter_context(nc.semaphore("s_dve"))
        s_st = [st.enter_context(nc.semaphore(f"s_st{b}")) for b in range(OB)]  # noqa: ANT232

        OPS = 7  # DVE tensor_tensor ops per tile

        @block.scalar
        def _(act):
            for i in range(n):
                t = i % nt
                if i >= IB:
                    j = i - IB
                    act.wait_ge(s_g[j % GB], 16 * (j // GB + 1))
                    act.wait_ge(s_dve, OPS * (j + 1))
                act.dma_start(
                    ibufs[i % IB][:, :], idx_d[:, t * cols_t : (t + 1) * cols_t]
                ).then_inc(s_ib[i % IB], 16)

        @block.gpsimd
        def _(gpsimd):
            gpsimd.load_library(mlp)
            for i in range(n):
                gpsimd.wait_ge(s_ib[i % IB], 16 * (i // IB + 1))
                if i >= GB:
                    gpsimd.wait_ge(s_dve, OPS * (i - GB + 1))
                g3 = gbufs[i % GB][:].rearrange("p (b e) -> p b e", e=512)
                gpsimd.dma_gather(
                    g3, tab_d, ibufs[i % IB][:, :cols_i], S, S, 512,
                    single_packet=False, queue_num=i % 4,
                ).then_inc(s_g[i % GB], 16)

        @block.vector
        def _(vector):
            for i in range(n):
                b, ob = i % GB, i % OB
                g3 = gbufs[b][:].rearrange("p (b e) -> p b e", e=512)
                o3 = obufs[ob][:].rearrange("p (b d) -> p b d", d=D)
                t3 = tbuf[:].rearrange("p (b d) -> p b d", d=D)

                def mk(r):
                    k = 3 - r
                    return (
                        ibufs[i % IB][:, cols_i + k * BLK : cols_i + (k + 1) * BLK]
                        .bitcast(mybir.dt.bfloat16)
                        .unsqueeze(2)
                        .to_broadcast([P, BLK, D])
                    )

                def gq(r):
                    return g3[:, :, r * D : (r + 1) * D]

                vector.wait_ge(s_g[b], 16 * (i // GB + 1))
                if i >= OB:
                    j = i - OB
                    vector.wait_ge(s_st[j % OB], 16 * (j // OB + 1))
                k = OPS * i
                vector.tensor_tensor(
                    out=o3, in0=gq(3), in1=mk(3), op=mybir.AluOpType.mult
                ).then_inc(s_dve, 1)
                for r in (2, 1, 0):
                    vector.wait_ge(s_dve, k + 1 + 2 * (2 - r))
                    vector.tensor_tensor(
                        out=t3, in0=gq(r), in1=mk(r), op=mybir.AluOpType.mult
                    ).then_inc(s_dve, 1)
                    vector.wait_ge(s_dve, k + 2 + 2 * (2 - r))
                    vector.tensor_tensor(
                        out=o3, in0=o3, in1=t3, op=mybir.AluOpType.add
                    ).then_inc(s_dve, 1)

        @block.sync
        def _(sync):
            for i in range(n):
                t = i % nt
                sync.wait_ge(s_dve, OPS * (i + 1))
                sync.dma_start(out_d[t], obufs[i % OB][:]).then_inc(
                    s_st[i % OB], 16
                )
            for b in range(OB):
                cnt = len([i for i in range(n) if i % OB == b])
                sync.wait_ge(s_st[b], 16 * cnt)

    nc.compile()
    return nc


import functools
import os

_BUILDERS = {
    "mod4": build_nc_mod4,
    "mod4raw": build_raw_mod4,
    "raw": build_raw,
    "chunk4": build_nc,
    "chunk4bf": functools.partial(build_nc, bf16=True),
}

VERSION = os.environ.get("KERNEL_VERSION", "chunk4bf")


def _get_nc():
    if "nc" not in _nc_cache:
        _nc_cache["nc"] = _BUILDERS[VERSION]()
    return _nc_cache["nc"]


def _prep_table(table, bf16=False):
    import ml_dtypes

    dt = ml_dtypes.bfloat16 if bf16 else np.float32
    t4 = np.zeros((NCH * TBL_CH, D), dt)
    for c in range(NCH):
        lo = c * V_CH
        hi = min(lo + V_CH, VOCAB)
        t4[c * TBL_CH : c * TBL_CH + (hi - lo)] = table[lo:hi].astype(dt)
    return t4


def _in_maps(input_batch, table, nt=NT, bf16=False):
    # slot i (within a tile) holds position p*BLK+b with p=i%128, b=i//128,
    # so dma_gather's i%128-partition placement lands rows in natural order
    # per partition. wrap-16: slot i sits at idx column i//16, partition i%16.
    v = np.asarray(input_batch).astype(np.int64).reshape(NCORES, NT, P, BLK)
    v = v[:, :nt]
    slots = v.transpose(0, 1, 3, 2).reshape(NCORES, nt, S)
    w = slots.reshape(NCORES, nt, S // 16, 16).transpose(0, 1, 3, 2)
    # zero-row target spread by slot id to avoid a single-address hotspot
    slotid = np.arange(S // 16)[None, :] * 16 + np.arange(16)[:, None]  # [16,256]
    zr = (V_CH + (slotid & (ZR - 1)))[None, None]
    chunks = [
        np.where((w >= 0) & (w // V_CH == c), w - c * V_CH, zr).astype(np.int16)
        for c in range(NCH)
    ]
    idx = np.stack(chunks, axis=2)              # [core, nt, NCH, 16, 256]
    # device layout: partition q (= q%16 replica), column t*NCH*256 + c*256 + s
    idx = idx.transpose(0, 3, 1, 2, 4)          # [core, 16, nt, NCH, 256]
    idx = idx.reshape(NCORES, 16, nt * NCH * (S // 16))
    idx = np.ascontiguousarray(np.tile(idx, (1, 8, 1)))   # replicate to 128
    t4 = _prep_table(np.asarray(table, dtype=np.float32), bf16=bf16)
    return [{"idx": idx[c], "table": t4} for c in range(NCORES)]


def _maps(input_batch, table):
    if VERSION in ("mod4", "mod4raw"):
        return _in_maps_mod4(input_batch, table)
    return _in_maps(input_batch, table, bf16=(VERSION == "chunk4bf"))


def build_for_bench(reps=1):
    return _BUILDERS[VERSION](reps=reps)


def kernel(input_batch, table):
    nc = _get_nc()
    in_maps = _maps(input_batch, table)
    res = run_bass_kernel_spmd(nc, in_maps, list(range(NCORES)))
    return np.concatenate(
        [
            res.results[c]["out"].reshape(NPOS // HIST, HIST, D)
            for c in range(NCORES)
        ],
        axis=0,
    )


def run_traced(input_batch, table, trace_cores=None, tmpdir=None):
    """Run once with NTFF profiling; returns (output, BassKernelResults)."""
    nc = _get_nc()
    in_maps = _maps(input_batch, table)
    res = run_bass_kernel_spmd(
        nc, in_maps, list(range(NCORES)), trace=True,
        trace_cores=trace_cores, tmpdir=tmpdir,
    )
    out = np.concatenate(
        [
            res.results[c]["out"].reshape(NPOS // HIST, HIST, D)
            for c in range(NCORES)
        ],
        axis=0,
    )
    return out, res


def bench(input_batch, table, reps=20, nc=None, chain=1):
    """Time repeated on-device executions (inputs device-resident, no
    donation, no host transfers in the timed region). Returns per-exec
    seconds (min over reps) which includes the axon dispatch round trip."""
    import time

    import jax
    from jax.sharding import Mesh, NamedSharding, PartitionSpec
    from jax.experimental.shard_map import shard_map

    from concourse import bass2jax
    from concourse.bass2jax import (
        _bass_exec_p,
        install_neuronx_cc_hook,
        partition_id_tensor,
    )

    if nc is None:
        nc = _get_nc()
    install_neuronx_cc_hook()
    in_maps = _maps(input_batch, table)

    partition_name = (
        nc.partition_id_tensor.name if nc.partition_id_tensor else None
    )
    in_names, out_names, out_avals, zero_outs = [], [], [], []
    for alloc in nc.m.functions[0].allocations:
        if not isinstance(alloc, mybir.MemoryLocationSet):
            continue
        name = alloc.memorylocations[0].name
        if alloc.kind == "ExternalInput":
            if name != partition_name:
                in_names.append(name)
        elif alloc.kind == "ExternalOutput":
            out_names.append(name)
            shape = tuple(alloc.tensor_shape)
            dtype = mybir.dt.np(alloc.dtype)
            out_avals.append(jax.core.ShapedArray(shape, dtype))
            zero_outs.append(np.zeros(shape, dtype))
    n_params = len(in_names)
    all_in_names = in_names + out_names
    if partition_name is not None:
        all_in_names = all_in_names + [partition_name]

    def _body(*args):
        ins_only = list(args[:n_params])
        outs = list(args[n_params:])
        pid = [partition_id_tensor()] if partition_name is not None else []
        for _ in range(chain):
            operands = ins_only + outs + pid
            outs = list(
                _bass_exec_p.bind(
                    *operands,
                    out_avals=tuple(out_avals),
                    in_names=tuple(all_in_names),
                    out_names=tuple(out_names),
                    lowering_input_output_aliases=(),
                    sim_require_finite=True,
                    sim_require_nnan=True,
                    nc=nc,
                )
            )
        return tuple(outs)

    devices = jax.devices()[:NCORES]
    mesh = Mesh(np.asarray(devices), ("core",))
    nshard = NamedSharding(mesh, PartitionSpec("core"))
    sharded = jax.jit(
        shard_map(
            _body,
            mesh=mesh,
            in_specs=(PartitionSpec("core"),) * (n_params + len(out_names)),
            out_specs=(PartitionSpec("core"),) * len(out_names),
            check_rep=False,
        ),
        keep_unused=True,
    )
    concat_in = [
        np.concatenate([np.asarray(in_maps[c][nm]) for c in range(NCORES)], axis=0)
        for nm in in_names
    ]
    concat_zeros = [
        np.zeros((NCORES * z.shape[0], *z.shape[1:]), z.dtype) for z in zero_outs
    ]
    dev_args = [jax.device_put(a, nshard) for a in concat_in + concat_zeros]
    jax.block_until_ready(dev_args)
    # warmup (compiles NEFF on first call)
    out = sharded(*dev_args)
    jax.block_until_ready(out)
    times = []
    for _ in range(reps):
        t0 = time.perf_counter()
        out = sharded(*dev_args)
        jax.block_until_ready(out)
        times.append(time.perf_counter() - t0)
    return min(times), times, out
